# revision 17
# baseline (speedup 1.0000x reference)
"""DocRED head, raw-bass (no TileContext) version.

Dataflow per core (batch element):
  gather   128 mention rows of hidden_states (indirect DMA, gpsimd queue)
  stage A  repT[h,e] = sum_m g[4e+m, h] -- 8 matmuls vs block-eye (fuses
           mention-sum + transpose)
  stage B  eL1 = rep @ G1, eL2 = rep @ G2 (G = dense_w @ out_w folded on
           host; fp16, split across the sync and tensor DMA queues)
  stage D  logits^T[c, p] = sum_e eLstack[e, c] * oh[e, p] -- 2 matmuls with
           the host-built one-hot stack (head rows 0-31, tail 32-63, ones 64)
           as the N=512 moving operand; beff rides eLstack row 64.
  out      [98, 1024] fp16, two DMA halves on the scalar and sync queues.

Raw semaphores; walrus's fixed end-of-program sem sweep does the cleanup.
"""

import numpy as np

import concourse.bass as bass
from concourse.bass_utils import run_bass_kernel_spmd
import concourse.bacc as bacc
import concourse.mybir as mybir

B, L, H, E, M, P, C = 8, 2048, 1024, 32, 4, 1024, 97
N_CORES = 8
HC = H // 128
CP = C + 1
NG = 2 * HC

f32 = mybir.dt.float32
f16 = mybir.dt.float16
i32 = mybir.dt.int32

ONES0 = 0            # lead cols 0..31: block-eye (stage A rhs)
EL0 = ONES0 + E      # lead cols 32..129: eL stack area (row 64 = beff)
LEADW = EL0 + CP + 2 # pad to even

N_WARM = 20          # PE clock-ramp dummies before stage A


def _build():
    nc = bacc.Bacc("TRN2", target_bir_lowering=False, debug=False)

    hs = nc.dram_tensor("hs", [L, H], f16, kind="ExternalInput").ap()
    pos = nc.dram_tensor("pos", [E * M, 1], i32, kind="ExternalInput").ap()
    lead = nc.dram_tensor("lead", [128, LEADW], f16, kind="ExternalInput").ap()
    oh = nc.dram_tensor("oh", [2 * E + 1, P], f16, kind="ExternalInput").ap()
    g = nc.dram_tensor("g", [128, NG * CP], f16, kind="ExternalInput").ap()
    out = nc.dram_tensor("out", [CP, P], f16, kind="ExternalOutput").ap()

    sb_pos = nc.alloc_sbuf_tensor("sb_pos", [E * M, 1], i32)
    sb_lead = nc.alloc_sbuf_tensor("sb_lead", [128, LEADW], f16)
    sb_oh = nc.alloc_sbuf_tensor("sb_oh", [2 * E + 1, P], f16)
    sb_G = nc.alloc_sbuf_tensor("sb_G", [128, NG * CP], f16)
    sb_g = nc.alloc_sbuf_tensor("sb_g", [E * M, H], f16)
    sb_repT = nc.alloc_sbuf_tensor("sb_repT", [128, HC * E], f16)
    sb_out = nc.alloc_sbuf_tensor("sb_out", [CP, P], f16)
    sb_warm = nc.alloc_sbuf_tensor("sb_warm", [128, E], f16)

    ps_w = nc.alloc_psum_tensor("ps_w", [E, E], f32)
    ps_a = nc.alloc_psum_tensor("ps_a", [128, HC * E], f32)
    ps_e1 = nc.alloc_psum_tensor("ps_e1", [E, CP], f32)
    ps_e2 = nc.alloc_psum_tensor("ps_e2", [E, CP], f32)
    ps_dq = [nc.alloc_psum_tensor(f"ps_d{q}", [CP, P // 4], f32)
             for q in range(4)]

    sPos = nc.alloc_semaphore("sPos")
    sLead = nc.alloc_semaphore("sLead")
    sOh = nc.alloc_semaphore("sOh")
    sG1 = nc.alloc_semaphore("sG1")
    sG2 = nc.alloc_semaphore("sG2")
    sGth = nc.alloc_semaphore("sGth")
    sW = nc.alloc_semaphore("sW")
    sA = nc.alloc_semaphore("sA")
    sRep = nc.alloc_semaphore("sRep")
    sE = nc.alloc_semaphore("sE")
    sEc = nc.alloc_semaphore("sEc")
    sD = nc.alloc_semaphore("sD")
    sOcpA = nc.alloc_semaphore("sOcpA")
    sOcpB = nc.alloc_semaphore("sOcpB")
    sOut = nc.alloc_semaphore("sOut")

    with nc.Block() as block:

        @block.gpsimd
        def _(gp):
            gp.wait_ge(sPos, 16)
            gp.indirect_dma_start(
                out=sb_g[:],
                out_offset=None,
                in_=hs[:],
                in_offset=bass.IndirectOffsetOnAxis(ap=sb_pos[:, :1], axis=0),
            ).then_inc(sGth, 16)
            gp.wait_ge(sOut, 64)

        @block.scalar
        def _(sc):
            sc.dma_start(sb_pos[:], pos[:]).then_inc(sPos, 16)
            sc.dma_start(sb_lead[:], lead[:]).then_inc(sLead, 16)
            sc.dma_start(sb_G[:, HC * CP:], g[:, HC * CP:]).then_inc(sG2, 16)
            sc.dma_start(sb_oh[:], oh[:]).then_inc(sOh, 16)
            sc.wait_ge(sE, 2)
            sc.copy(out=sb_lead[E:2 * E, EL0:EL0 + CP], in_=ps_e2[:]).then_inc(
                sEc, 1)
            PQ = P // 4
            sc.wait_ge(sD, 2)
            sc.copy(out=sb_out[:, PQ:2 * PQ], in_=ps_dq[1][:]).then_inc(
                sOcpB, 1)
            sc.wait_ge(sOcpA, 1)
            sc.dma_start(out[:, :PQ], sb_out[:, :PQ]).then_inc(sOut, 16)
            sc.wait_ge(sD, 4)
            sc.copy(out=sb_out[:, 3 * PQ:], in_=ps_dq[3][:]).then_inc(
                sOcpB, 1)
            sc.wait_ge(sOcpA, 2)
            sc.dma_start(out[:, 2 * PQ:3 * PQ],
                         sb_out[:, 2 * PQ:3 * PQ]).then_inc(sOut, 16)

        @block.sync
        def _(sy):
            sy.dma_start(sb_G[:, : HC * CP], g[:, : HC * CP]).then_inc(sG1, 16)
            PQ = P // 4
            sy.wait_ge(sOcpB, 1)
            sy.dma_start(out[:, PQ:2 * PQ], sb_out[:, PQ:2 * PQ]).then_inc(
                sOut, 16)
            sy.wait_ge(sOcpB, 2)
            sy.dma_start(out[:, 3 * PQ:], sb_out[:, 3 * PQ:]).then_inc(sOut, 16)

        @block.vector
        def _(ve):
            ve.memset(sb_warm[:], 0.0).then_inc(sW, 1)
            ve.wait_ge(sA, 1)
            ve.tensor_copy(out=sb_repT[:], in_=ps_a[:]).then_inc(sRep, 1)
            ve.wait_ge(sLead, 16)  # eL stack area overlays the lead tile
            ve.wait_ge(sE, 1)
            ve.tensor_copy(
                out=sb_lead[:E, EL0:EL0 + CP], in_=ps_e1[:]
            ).then_inc(sEc, 1)
            PQ = P // 4
            ve.wait_ge(sD, 1)
            ve.tensor_copy(out=sb_out[:, :PQ], in_=ps_dq[0][:]).then_inc(
                sOcpA, 1)
            ve.wait_ge(sD, 3)
            ve.tensor_copy(
                out=sb_out[:, 2 * PQ:3 * PQ], in_=ps_dq[2][:]
            ).then_inc(sOcpA, 1)

        @block.tensor
        def _(te):
            # clock-ramp dummies while the gather runs
            te.wait_ge(sW, 1)
            for _i in range(N_WARM):
                te.matmul(
                    out=ps_w[:], lhsT=sb_warm[:, :E], rhs=sb_warm[:, :E],
                    start=True, stop=True,
                )
            te.wait_ge(sLead, 16)
            te.wait_ge(sGth, 16)
            for hc in range(HC):
                mm = te.matmul(
                    out=ps_a[:, hc * E:(hc + 1) * E],
                    lhsT=sb_g[:, hc * 128:(hc + 1) * 128],
                    rhs=sb_lead[:, ONES0:ONES0 + E],
                    start=True,
                    stop=True,
                )
            mm.then_inc(sA, 1)
            te.wait_ge(sRep, 1)
            te.wait_ge(sG1, 16)
            for hc in range(HC):
                mm = te.matmul(
                    out=ps_e1[:],
                    lhsT=sb_repT[:, hc * E:(hc + 1) * E],
                    rhs=sb_G[:, hc * CP:(hc + 1) * CP],
                    start=(hc == 0),
                    stop=(hc == HC - 1),
                )
            mm.then_inc(sE, 1)
            te.wait_ge(sG2, 16)
            for hc in range(HC):
                mm = te.matmul(
                    out=ps_e2[:],
                    lhsT=sb_repT[:, hc * E:(hc + 1) * E],
                    rhs=sb_G[:, (HC + hc) * CP:(HC + hc + 1) * CP],
                    start=(hc == 0),
                    stop=(hc == HC - 1),
                )
            mm.then_inc(sE, 1)
            te.wait_ge(sEc, 2)
            te.wait_ge(sOh, 16)
            PQ = P // 4
            for q in range(4):
                te.matmul(
                    out=ps_dq[q][:],
                    lhsT=sb_lead[: 2 * E + 1, EL0:EL0 + CP],
                    rhs=sb_oh[:, q * PQ:(q + 1) * PQ],
                    start=True,
                    stop=True,
                ).then_inc(sD, 1)

    nc.compile()
    return nc


def make_in_maps(hidden_states, dense_w, dense_b, out_w, out_b,
                 entity_position_ids, head_tail_idxs):
    hidden_states = np.asarray(hidden_states)
    dense_w = np.asarray(dense_w, dtype=np.float32)
    dense_b = np.asarray(dense_b, dtype=np.float32)
    out_w = np.asarray(out_w, dtype=np.float32)
    out_b = np.asarray(out_b, dtype=np.float32)
    entity_position_ids = np.asarray(entity_position_ids)
    head_tail_idxs = np.asarray(head_tail_idxs)

    gfull = np.zeros((2 * H, CP), np.float32)
    gfull[:, :C] = dense_w @ out_w
    g16 = np.ascontiguousarray(
        gfull.astype(np.float16).reshape(NG, 128, CP)
        .transpose(1, 0, 2).reshape(128, NG * CP))
    beff = dense_b @ out_w + out_b  # [97]

    lead = np.zeros((128, LEADW), np.float16)
    lead[:, ONES0:ONES0 + E] = np.repeat(np.eye(E, dtype=np.float16), M, axis=0)
    lead[2 * E, EL0:EL0 + C] = beff.astype(np.float16)

    iota = np.arange(E, dtype=np.int32)
    in_maps = []
    for b in range(B):
        ht = head_tail_idxs[b]  # [P, 2] int32
        ohm = np.zeros((2 * E + 1, P), np.float16)
        ohm[:E, :] = (ht[None, :, 0] == iota[:, None])
        ohm[E:2 * E, :] = (ht[None, :, 1] == iota[:, None])
        ohm[2 * E, :] = 1.0
        in_maps.append({
            "hs": np.ascontiguousarray(hidden_states[b], dtype=np.float16),
            "pos": np.ascontiguousarray(
                entity_position_ids[b].reshape(E * M, 1).astype(np.int32)),
            "lead": lead,
            "oh": ohm,
            "g": g16,
        })
    return in_maps


def postprocess(res_out):
    # device out is [98, 1024] f16: row c = logits[:, c]
    return np.ascontiguousarray(res_out[:C].T.astype(np.float32))


_CACHE = {}


def get_compiled():
    if "nc" not in _CACHE:
        _CACHE["nc"] = _build()
    return _CACHE["nc"]


def kernel(hidden_states, dense_w, dense_b, out_w, out_b,
           entity_position_ids, head_tail_idxs, _trace=False, _trace_kwargs=None):
    nc = get_compiled()
    in_maps = make_in_maps(hidden_states, dense_w, dense_b, out_w, out_b,
                           entity_position_ids, head_tail_idxs)
    res = run_bass_kernel_spmd(
        nc, in_maps, core_ids=list(range(N_CORES)),
        trace=_trace, **(_trace_kwargs or {}),
    )
    outp = np.concatenate(
        [np.ascontiguousarray(res.results[i]["out"][:C].T.astype(np.float32))
         for i in range(N_CORES)], axis=0)
    if _trace:
        return outp, res
    return outp
